# revision 18
# baseline (speedup 1.0000x reference)
"""Trainium2 Bass kernel for nn_EdgeDecoder (GNN edge decoder, 2 relations).

Strategy (data-parallel over edges, 8 NeuronCores):
  - Host pre-gathers the per-edge embedding rows and stores them TRANSPOSED
    ([128 dim, E_core] fp16) in DRAM, one pair of tensors per relation.
    The device kernel is a pure dense pipeline; per 1024-edge unit (2 PSUM
    supertiles, one per hidden half):
      pre  = W1u^T u + W1v^T v   (fp16 matmuls, f32 PSUM [128,1024], with
                                  same-lhsT matmuls adjacent so the PE
                                  reloads stationary weights half as often)
      ht   = relu(pre + b1)      (one wide Act instr per supertile)
    W2 dot-products for three 512-edge tiles pack into ONE PSUM bank at
    partition positions {0,32,64} (tile_position), with lhsT grouped
    w2a*3 then w2b*3 (2 weight loads per 3 tiles). DVE copies the packed
    bank to SBUF; DMA ships [128,512] f32 per group and the host slices
    rows {0,32,64}. b2 is added on host.
"""
import sys

if "/opt/trn_rl_repo" not in sys.path:
    sys.path.insert(0, "/opt/trn_rl_repo")

import numpy as np

P = 128
D = 128
HID = 256
E = 500000
NCORES = 8
EPC = E // NCORES          # 62500 edges per core per relation
CCH = 512                  # edges per W2 matmul (one PSUM bank of f32)
UCH = 1024                 # edges per W1 supertile / Act instr
NREL = 2
PADL = 62976               # EPC padded to a multiple of CCH (123 cc tiles)
NCC = PADL // CCH          # 123 compute tiles per relation
NG = -(-NCC // 3)          # 41 logit groups (3 cc per PSUM bank) per rel
GCH = 8192                 # edges per DMA slab
# small leading slabs so the PE starts sooner after the first DMA; the
# last slab carries the odd trailing 512-edge half-unit
_sizes = [1024, 1024, 4096] + [8192] * 6 + [7680]
CHUNKS = []
_o = 0
for _s in _sizes:
    CHUNKS.append((_o, _s))
    _o += _s
assert _o == PADL

_PROGRAM_CACHE = {}
LAST_RESULTS = None


def _build_program():
    import concourse.bacc as bacc
    import concourse.bass as bass
    import concourse.mybir as mybir
    from concourse.tile import TileContext

    f16, f32 = mybir.dt.float16, mybir.dt.float32
    relu = mybir.ActivationFunctionType.Relu

    nc = bacc.Bacc("TRN2", target_bir_lowering=False, debug=False)

    ut_d = [nc.dram_tensor(f"ut{r}", [P, PADL], f16, kind="ExternalInput")
            for r in range(NREL)]
    vt_d = [nc.dram_tensor(f"vt{r}", [P, PADL], f16, kind="ExternalInput")
            for r in range(NREL)]
    w1u_d = [nc.dram_tensor(f"w1u{r}", [D, HID], f16, kind="ExternalInput")
             for r in range(NREL)]
    w1v_d = [nc.dram_tensor(f"w1v{r}", [D, HID], f16, kind="ExternalInput")
             for r in range(NREL)]
    w2_d = [nc.dram_tensor(f"w2{r}", [P, 2], f16, kind="ExternalInput")
            for r in range(NREL)]
    b1_d = [nc.dram_tensor(f"b1{r}", [P, 2], f32, kind="ExternalInput")
            for r in range(NREL)]
    outs = [nc.dram_tensor(f"o{r}", [NG, 3, CCH], f32, kind="ExternalOutput")
            for r in range(NREL)]

    with TileContext(nc) as tc:
        with tc.tile_pool(name="sbw", bufs=1) as sbw, \
             tc.tile_pool(name="sbu", bufs=3) as sbu, \
             tc.tile_pool(name="sbv", bufs=3) as sbv, \
             tc.tile_pool(name="sbh", bufs=8) as sbh, \
             tc.tile_pool(name="sblog", bufs=2) as sblog, \
             tc.tile_pool(name="ph", bufs=3, space="PSUM") as ph, \
             tc.tile_pool(name="pl", bufs=2, space="PSUM") as pl:

            # first slab pair of relation 0 is on the critical path: issue
            # it on both HWDGE queues (u on SP, v on Act) BEFORE the weight
            # loads so the PE can start ~5us sooner
            first_gu = sbu.tile([P, GCH], f16, tag="gu")
            nc.sync.dma_start(out=first_gu[:, :CHUNKS[0][1]],
                              in_=ut_d[0].ap()[:, 0:CHUNKS[0][1]])
            first_gv = sbv.tile([P, GCH], f16, tag="gv")
            nc.scalar.dma_start(out=first_gv[:, :CHUNKS[0][1]],
                                in_=vt_d[0].ap()[:, 0:CHUNKS[0][1]])

            w1u_t, w1v_t, w2_t, b1_t = [], [], [], []
            for r in range(NREL):
                t = sbw.tile([D, HID], f16, tag=f"w1u{r}")
                nc.sync.dma_start(out=t[:], in_=w1u_d[r].ap()[:])
                w1u_t.append(t)
                t = sbw.tile([D, HID], f16, tag=f"w1v{r}")
                nc.sync.dma_start(out=t[:], in_=w1v_d[r].ap()[:])
                w1v_t.append(t)
                t = sbw.tile([P, 2], f16, tag=f"w2{r}")
                nc.scalar.dma_start(out=t[:], in_=w2_d[r].ap()[:])
                w2_t.append(t)
                t = sbw.tile([P, 2], f32, tag=f"b1{r}")
                nc.scalar.dma_start(out=t[:], in_=b1_d[r].ap()[:])
                b1_t.append(t)

            # queue of finished (ht0, ht1, col-slice) per 512-edge cc tile;
            # drained three-at-a-time into one packed PSUM logit bank. The
            # >=5 threshold keeps flushed ccs at least one 1024-edge unit
            # behind the W1 matmuls, hiding the relu round-trip on Act.
            POS = (0, 32, 64)
            relu_rr = 0

            def flush(r_, q_, g_, n_):
                ccs = [q_.pop(0) for _ in range(n_)]
                plb = pl.tile([P, CCH], f32, tag="pl")
                for hsel in range(2):
                    for i, (ht0_, ht1_, sl_) in enumerate(ccs):
                        h = ht0_ if hsel == 0 else ht1_
                        nc.tensor.matmul(out=plb[POS[i]:POS[i] + 1, :],
                                         lhsT=w2_t[r_][:, hsel:hsel + 1],
                                         rhs=h[:, sl_],
                                         start=(hsel == 0),
                                         stop=(hsel == 1))
                logt = sblog.tile([P, CCH], f32, tag="log")
                nc.vector.tensor_copy(out=logt[:], in_=plb[:])
                lap = logt[:]
                strided = bass.AP(lap.tensor, lap.offset,
                                  [[32 * CCH, 3], [1, CCH]])
                nc.sync.dma_start(out=outs[r_].ap()[g_], in_=strided)

            for r in range(NREL):
                queue = []
                gidx = 0
                for ci, (off, csz) in enumerate(CHUNKS):
                    if r == 0 and ci == 0:
                        gu, gv = first_gu, first_gv
                    else:
                        # u slabs issue from the SP HWDGE queue, v slabs
                        # from the Act queue: parallel DMA issue halves the
                        # slab-pair arrival latency
                        gu = sbu.tile([P, GCH], f16, tag="gu")
                        nc.sync.dma_start(out=gu[:, :csz],
                                          in_=ut_d[r].ap()[:, off:off + csz])
                        gv = sbv.tile([P, GCH], f16, tag="gv")
                        nc.scalar.dma_start(out=gv[:, :csz],
                                            in_=vt_d[r].ap()[:, off:off + csz])
                    for unit in range(-(-csz // UCH)):
                        ub = unit * UCH
                        usz = min(UCH, csz - ub)     # 1024, or 512 tail
                        sls = [slice(ub + j * CCH, ub + (j + 1) * CCH)
                               for j in range(usz // CCH)]
                        ph0 = ph.tile([P, UCH], f32, tag="ph")
                        ph1 = ph.tile([P, UCH], f32, tag="ph")
                        # grouped by stationary operand: 2 matmuls per
                        # PE weight load instead of 1
                        for j, sl in enumerate(sls):
                            nc.tensor.matmul(
                                out=ph0[:, j * CCH:(j + 1) * CCH],
                                lhsT=w1u_t[r][:, 0:P], rhs=gu[:, sl],
                                start=True, stop=False)
                        for j, sl in enumerate(sls):
                            nc.tensor.matmul(
                                out=ph0[:, j * CCH:(j + 1) * CCH],
                                lhsT=w1v_t[r][:, 0:P], rhs=gv[:, sl],
                                start=False, stop=True)
                        for j, sl in enumerate(sls):
                            nc.tensor.matmul(
                                out=ph1[:, j * CCH:(j + 1) * CCH],
                                lhsT=w1u_t[r][:, P:2 * P], rhs=gu[:, sl],
                                start=True, stop=False)
                        for j, sl in enumerate(sls):
                            nc.tensor.matmul(
                                out=ph1[:, j * CCH:(j + 1) * CCH],
                                lhsT=w1v_t[r][:, P:2 * P], rhs=gv[:, sl],
                                start=False, stop=True)
                        # relu + bias: mostly on Act; every 4th half-tile on
                        # DVE (fused add-bias + max-0 tensor_scalar) to keep
                        # the two engines' busy times balanced
                        ht0 = sbh.tile([P, UCH], f16, tag="ht")
                        ht1 = sbh.tile([P, UCH], f16, tag="ht")
                        for hsel, (pht, htt) in enumerate(((ph0, ht0),
                                                          (ph1, ht1))):
                            if relu_rr % 4 == 3:
                                nc.vector.tensor_scalar(
                                    out=htt[:, :usz], in0=pht[:, :usz],
                                    scalar1=b1_t[r][:, hsel:hsel + 1],
                                    scalar2=0.0,
                                    op0=mybir.AluOpType.add,
                                    op1=mybir.AluOpType.max)
                            else:
                                nc.scalar.activation(
                                    out=htt[:, :usz], in_=pht[:, :usz],
                                    func=relu,
                                    bias=b1_t[r][:, hsel:hsel + 1])
                            relu_rr += 1
                        queue.append((ht0, ht1, slice(0, CCH)))
                        if usz == UCH:
                            queue.append((ht0, ht1, slice(CCH, UCH)))
                        while len(queue) >= 5:
                            flush(r, queue, gidx, 3)
                            gidx += 1
                while queue:
                    flush(r, queue, gidx, min(3, len(queue)))
                    gidx += 1
    nc.compile()
    return nc


def _prep(user_embed, item_embed, u_clicks, v_clicks, u_buys, v_buys,
          W1_clicks, b1_clicks, W2_clicks, b2_clicks,
          W1_buys, b1_buys, W2_buys, b2_buys):
    user16 = np.asarray(user_embed, np.float32).astype(np.float16)
    item16 = np.asarray(item_embed, np.float32).astype(np.float16)
    rels = [
        (np.asarray(u_clicks), np.asarray(v_clicks),
         np.asarray(W1_clicks, np.float32), np.asarray(b1_clicks, np.float32),
         np.asarray(W2_clicks, np.float32), np.asarray(b2_clicks, np.float32)),
        (np.asarray(u_buys), np.asarray(v_buys),
         np.asarray(W1_buys, np.float32), np.asarray(b1_buys, np.float32),
         np.asarray(W2_buys, np.float32), np.asarray(b2_buys, np.float32)),
    ]

    in_maps = [dict() for _ in range(NCORES)]
    b2s = []
    for r, (u_all, v_all, W1, b1, W2, b2) in enumerate(rels):
        b2s.append(float(b2[0]))
        w1u = W1[:D].astype(np.float16)
        w1v = W1[D:].astype(np.float16)
        w2 = W2.reshape(2, P).T.astype(np.float16).copy()
        b1m = b1.reshape(2, P).T.astype(np.float32).copy()
        gu = user16[u_all]                     # [E, 128] f16
        gv = item16[v_all]
        for k in range(NCORES):
            m = in_maps[k]
            m[f"w1u{r}"] = w1u
            m[f"w1v{r}"] = w1v
            m[f"w2{r}"] = w2
            m[f"b1{r}"] = b1m
            buf = np.zeros((P, PADL), np.float16)
            buf[:, :EPC] = gu[k * EPC:(k + 1) * EPC].T
            m[f"ut{r}"] = buf
            buf = np.zeros((P, PADL), np.float16)
            buf[:, :EPC] = gv[k * EPC:(k + 1) * EPC].T
            m[f"vt{r}"] = buf
    return in_maps, b2s


def make_in_maps(np_inputs):
    """For external harnesses: per-core input maps for the cached program."""
    return _prep(**np_inputs)[0]


def kernel(**inputs):
    global LAST_RESULTS
    from concourse import bass_utils

    in_maps, b2s = _prep(**inputs)

    if "prog" not in _PROGRAM_CACHE:
        _PROGRAM_CACHE["prog"] = _build_program()
    nc = _PROGRAM_CACHE["prog"]

    res = bass_utils.run_bass_kernel_spmd(nc, in_maps, core_ids=list(range(NCORES)))
    LAST_RESULTS = res

    outs = []
    for r in range(NREL):
        full = np.empty(E, np.float32)
        for k in range(NCORES):
            o = res.results[k][f"o{r}"]                  # [NG, 3, 512]
            flat = o.reshape(-1)[:PADL]
            full[k * EPC:(k + 1) * EPC] = flat[:EPC]
        full += b2s[r]
        outs.append(full)
    return outs[0], outs[1]


# revision 20
# speedup vs baseline: 1.0951x; 1.0951x over previous
"""Trainium2 Bass kernel for nn_EdgeDecoder (GNN edge decoder, 2 relations).

Strategy (data-parallel over edges, 8 NeuronCores):
  - Host pre-gathers the per-edge embedding rows and stores them TRANSPOSED
    ([128 dim, E_core] fp16) in DRAM, one pair of tensors per relation.
    The device kernel is a pure dense pipeline; per 1024-edge unit (2 PSUM
    supertiles, one per hidden half):
      pre  = W1u^T u + W1v^T v   (fp16 matmuls, f32 PSUM [128,1024], with
                                  same-lhsT matmuls adjacent so the PE
                                  reloads stationary weights half as often)
      ht   = relu(pre + b1)      (one wide Act instr per supertile)
    W2 dot-products for three 512-edge tiles pack into ONE PSUM bank at
    partition positions {0,32,64} (tile_position), with lhsT grouped
    w2a*3 then w2b*3 (2 weight loads per 3 tiles). DVE copies the packed
    bank to SBUF; DMA ships [128,512] f32 per group and the host slices
    rows {0,32,64}. b2 is added on host.
"""
import sys

if "/opt/trn_rl_repo" not in sys.path:
    sys.path.insert(0, "/opt/trn_rl_repo")

import numpy as np

P = 128
D = 128
HID = 256
E = 500000
NCORES = 8
EPC = E // NCORES          # 62500 edges per core per relation
CCH = 512                  # edges per W2 matmul (one PSUM bank of f32)
UCH = 1024                 # edges per W1 supertile / Act instr
NREL = 2
PADL = 62976               # EPC padded to a multiple of CCH (123 cc tiles)
NCC = PADL // CCH          # 123 compute tiles per relation
NG = -(-NCC // 3)          # 41 logit groups (3 cc per PSUM bank) per rel
GCH = 8192                 # edges per DMA slab
# small leading slabs so the PE starts sooner after the first DMA; the
# last slab carries the odd trailing 512-edge half-unit
_sizes = [1024, 1024, 4096] + [8192] * 6 + [7680]
CHUNKS = []
_o = 0
for _s in _sizes:
    CHUNKS.append((_o, _s))
    _o += _s
assert _o == PADL

_PROGRAM_CACHE = {}
LAST_RESULTS = None


def _build_program():
    import concourse.bacc as bacc
    import concourse.bass as bass
    import concourse.mybir as mybir
    from concourse.tile import TileContext

    f16, f32 = mybir.dt.float16, mybir.dt.float32
    relu = mybir.ActivationFunctionType.Relu

    nc = bacc.Bacc("TRN2", target_bir_lowering=False, debug=False)

    ut_d = [nc.dram_tensor(f"ut{r}", [P, PADL], f16, kind="ExternalInput")
            for r in range(NREL)]
    vt_d = [nc.dram_tensor(f"vt{r}", [P, PADL], f16, kind="ExternalInput")
            for r in range(NREL)]
    w1u_d = [nc.dram_tensor(f"w1u{r}", [D, HID], f16, kind="ExternalInput")
             for r in range(NREL)]
    w1v_d = [nc.dram_tensor(f"w1v{r}", [D, HID], f16, kind="ExternalInput")
             for r in range(NREL)]
    w2_d = [nc.dram_tensor(f"w2{r}", [P, 2], f16, kind="ExternalInput")
            for r in range(NREL)]
    b1_d = [nc.dram_tensor(f"b1{r}", [P, 2], f32, kind="ExternalInput")
            for r in range(NREL)]
    outs = [nc.dram_tensor(f"o{r}", [NG, 3, CCH], f32, kind="ExternalOutput")
            for r in range(NREL)]

    with TileContext(nc) as tc:
        with tc.tile_pool(name="sbw", bufs=1) as sbw, \
             tc.tile_pool(name="sbu", bufs=3) as sbu, \
             tc.tile_pool(name="sbv", bufs=3) as sbv, \
             tc.tile_pool(name="sbh", bufs=8) as sbh, \
             tc.tile_pool(name="sblog", bufs=2) as sblog, \
             tc.tile_pool(name="ph", bufs=3, space="PSUM") as ph, \
             tc.tile_pool(name="pl", bufs=2, space="PSUM") as pl:

            # first slab pair of relation 0 is on the critical path: issue
            # it BEFORE the weight loads so the PE can start sooner
            first_gu = sbu.tile([P, GCH], f16, tag="gu")
            nc.sync.dma_start(out=first_gu[:, :CHUNKS[0][1]],
                              in_=ut_d[0].ap()[:, 0:CHUNKS[0][1]])
            first_gv = sbv.tile([P, GCH], f16, tag="gv")
            nc.sync.dma_start(out=first_gv[:, :CHUNKS[0][1]],
                              in_=vt_d[0].ap()[:, 0:CHUNKS[0][1]])

            w1u_t, w1v_t, w2_t, b1_t = [], [], [], []
            for r in range(NREL):
                t = sbw.tile([D, HID], f16, tag=f"w1u{r}")
                nc.sync.dma_start(out=t[:], in_=w1u_d[r].ap()[:])
                w1u_t.append(t)
                t = sbw.tile([D, HID], f16, tag=f"w1v{r}")
                nc.sync.dma_start(out=t[:], in_=w1v_d[r].ap()[:])
                w1v_t.append(t)
                t = sbw.tile([P, 2], f16, tag=f"w2{r}")
                nc.sync.dma_start(out=t[:], in_=w2_d[r].ap()[:])
                w2_t.append(t)
                t = sbw.tile([P, 2], f32, tag=f"b1{r}")
                nc.sync.dma_start(out=t[:], in_=b1_d[r].ap()[:])
                b1_t.append(t)

            # queue of finished (ht0, ht1, col-slice) per 512-edge cc tile;
            # drained three-at-a-time into one packed PSUM logit bank. The
            # >=5 threshold keeps flushed ccs at least one 1024-edge unit
            # behind the W1 matmuls, hiding the relu round-trip on Act.
            POS = (0, 32, 64)
            relu_rr = 0

            def flush(r_, q_, g_, n_):
                ccs = [q_.pop(0) for _ in range(n_)]
                plb = pl.tile([P, CCH], f32, tag="pl")
                for hsel in range(2):
                    for i, (ht0_, ht1_, sl_) in enumerate(ccs):
                        h = ht0_ if hsel == 0 else ht1_
                        nc.tensor.matmul(out=plb[POS[i]:POS[i] + 1, :],
                                         lhsT=w2_t[r_][:, hsel:hsel + 1],
                                         rhs=h[:, sl_],
                                         start=(hsel == 0),
                                         stop=(hsel == 1))
                logt = sblog.tile([P, CCH], f32, tag="log")
                nc.vector.tensor_copy(out=logt[:], in_=plb[:])
                lap = logt[:]
                strided = bass.AP(lap.tensor, lap.offset,
                                  [[32 * CCH, 3], [1, CCH]])
                nc.sync.dma_start(out=outs[r_].ap()[g_], in_=strided)

            for r in range(NREL):
                queue = []
                gidx = 0
                for ci, (off, csz) in enumerate(CHUNKS):
                    if r == 0 and ci == 0:
                        gu, gv = first_gu, first_gv
                    else:
                        gu = sbu.tile([P, GCH], f16, tag="gu")
                        nc.sync.dma_start(out=gu[:, :csz],
                                          in_=ut_d[r].ap()[:, off:off + csz])
                        gv = sbv.tile([P, GCH], f16, tag="gv")
                        nc.sync.dma_start(out=gv[:, :csz],
                                          in_=vt_d[r].ap()[:, off:off + csz])
                    for unit in range(-(-csz // UCH)):
                        ub = unit * UCH
                        usz = min(UCH, csz - ub)     # 1024, or 512 tail
                        sls = [slice(ub + j * CCH, ub + (j + 1) * CCH)
                               for j in range(usz // CCH)]
                        ph0 = ph.tile([P, UCH], f32, tag="ph")
                        ph1 = ph.tile([P, UCH], f32, tag="ph")
                        # grouped by stationary operand: 2 matmuls per
                        # PE weight load instead of 1
                        for j, sl in enumerate(sls):
                            nc.tensor.matmul(
                                out=ph0[:, j * CCH:(j + 1) * CCH],
                                lhsT=w1u_t[r][:, 0:P], rhs=gu[:, sl],
                                start=True, stop=False)
                        for j, sl in enumerate(sls):
                            nc.tensor.matmul(
                                out=ph0[:, j * CCH:(j + 1) * CCH],
                                lhsT=w1v_t[r][:, 0:P], rhs=gv[:, sl],
                                start=False, stop=True)
                        for j, sl in enumerate(sls):
                            nc.tensor.matmul(
                                out=ph1[:, j * CCH:(j + 1) * CCH],
                                lhsT=w1u_t[r][:, P:2 * P], rhs=gu[:, sl],
                                start=True, stop=False)
                        for j, sl in enumerate(sls):
                            nc.tensor.matmul(
                                out=ph1[:, j * CCH:(j + 1) * CCH],
                                lhsT=w1v_t[r][:, P:2 * P], rhs=gv[:, sl],
                                start=False, stop=True)
                        # relu + bias: mostly on Act; every 4th half-tile on
                        # DVE (fused add-bias + max-0 tensor_scalar) to keep
                        # the two engines' busy times balanced
                        ht0 = sbh.tile([P, UCH], f16, tag="ht")
                        ht1 = sbh.tile([P, UCH], f16, tag="ht")
                        for hsel, (pht, htt) in enumerate(((ph0, ht0),
                                                          (ph1, ht1))):
                            if relu_rr % 4 == 3:
                                nc.vector.tensor_scalar(
                                    out=htt[:, :usz], in0=pht[:, :usz],
                                    scalar1=b1_t[r][:, hsel:hsel + 1],
                                    scalar2=0.0,
                                    op0=mybir.AluOpType.add,
                                    op1=mybir.AluOpType.max)
                            else:
                                nc.scalar.activation(
                                    out=htt[:, :usz], in_=pht[:, :usz],
                                    func=relu,
                                    bias=b1_t[r][:, hsel:hsel + 1])
                            relu_rr += 1
                        queue.append((ht0, ht1, slice(0, CCH)))
                        if usz == UCH:
                            queue.append((ht0, ht1, slice(CCH, UCH)))
                        while len(queue) >= 5:
                            flush(r, queue, gidx, 3)
                            gidx += 1
                while queue:
                    flush(r, queue, gidx, min(3, len(queue)))
                    gidx += 1
    nc.compile()
    return nc


def _prep(user_embed, item_embed, u_clicks, v_clicks, u_buys, v_buys,
          W1_clicks, b1_clicks, W2_clicks, b2_clicks,
          W1_buys, b1_buys, W2_buys, b2_buys):
    user16 = np.asarray(user_embed, np.float32).astype(np.float16)
    item16 = np.asarray(item_embed, np.float32).astype(np.float16)
    rels = [
        (np.asarray(u_clicks), np.asarray(v_clicks),
         np.asarray(W1_clicks, np.float32), np.asarray(b1_clicks, np.float32),
         np.asarray(W2_clicks, np.float32), np.asarray(b2_clicks, np.float32)),
        (np.asarray(u_buys), np.asarray(v_buys),
         np.asarray(W1_buys, np.float32), np.asarray(b1_buys, np.float32),
         np.asarray(W2_buys, np.float32), np.asarray(b2_buys, np.float32)),
    ]

    in_maps = [dict() for _ in range(NCORES)]
    b2s = []
    for r, (u_all, v_all, W1, b1, W2, b2) in enumerate(rels):
        b2s.append(float(b2[0]))
        w1u = W1[:D].astype(np.float16)
        w1v = W1[D:].astype(np.float16)
        w2 = W2.reshape(2, P).T.astype(np.float16).copy()
        b1m = b1.reshape(2, P).T.astype(np.float32).copy()
        gu = user16[u_all]                     # [E, 128] f16
        gv = item16[v_all]
        for k in range(NCORES):
            m = in_maps[k]
            m[f"w1u{r}"] = w1u
            m[f"w1v{r}"] = w1v
            m[f"w2{r}"] = w2
            m[f"b1{r}"] = b1m
            buf = np.zeros((P, PADL), np.float16)
            buf[:, :EPC] = gu[k * EPC:(k + 1) * EPC].T
            m[f"ut{r}"] = buf
            buf = np.zeros((P, PADL), np.float16)
            buf[:, :EPC] = gv[k * EPC:(k + 1) * EPC].T
            m[f"vt{r}"] = buf
    return in_maps, b2s


def make_in_maps(np_inputs):
    """For external harnesses: per-core input maps for the cached program."""
    return _prep(**np_inputs)[0]


def kernel(**inputs):
    global LAST_RESULTS
    from concourse import bass_utils

    in_maps, b2s = _prep(**inputs)

    if "prog" not in _PROGRAM_CACHE:
        _PROGRAM_CACHE["prog"] = _build_program()
    nc = _PROGRAM_CACHE["prog"]

    res = bass_utils.run_bass_kernel_spmd(nc, in_maps, core_ids=list(range(NCORES)))
    LAST_RESULTS = res

    outs = []
    for r in range(NREL):
        full = np.empty(E, np.float32)
        for k in range(NCORES):
            o = res.results[k][f"o{r}"]                  # [NG, 3, 512]
            flat = o.reshape(-1)[:PADL]
            full[k * EPC:(k + 1) * EPC] = flat[:EPC]
        full += b2s[r]
        outs.append(full)
    return outs[0], outs[1]
